# revision 26
# baseline (speedup 1.0000x reference)
"""MQA attention (B=2, Lq=Lkv=2048, F=1024, H=16, D=64) on 8 TRN2 cores.

Sharding: core = (batch, query-block-of-512). Each core computes its full
output rows (all 16 heads + output projection) -> no collectives; host
concatenates per-core yT slabs.

v2 (pipelined): fp16 inputs/weights, KV projection first, then an
ACT-bound software pipeline over 8 head-pairs where pair j+1's
q-projection + RoPE are emitted inside pair j's attention chunk loop.
Mask multiplies alternate DVE/GpSimd; softmax denominators use
reciprocal_approx_fast + DMA partition-broadcast off the critical path.

Per-core dataflow:
  kvT[kd|vd,lk] = Wkv.T @ xkvT          (fp16 x, fp16 w, fp32 psum)
  RoPE in halves-permuted basis (host permutes Wq/Wk columns):
  x_rot = x*cos + Swap @ (x*sin_signed), Swap = permutation on the PE.
  qT[hd,lq] per pair = Wq'.T @ xqT  (+RoPE, scaled 1/sqrt(D) via tables)
  S^T[lk,lq] per head = k-chunk.T @ qT   (fp16, zero-padded K=128)
  P^T = exp(S^T) * maskT  (ACT exp -> fp16; mask mul on DVE/GpSimd)
  O_aug^T = V_aug-chunk.T @ P^T  (ones column -> row 64 = denominator)
  normalize: reciprocal_approx_fast(Z) -> DMA broadcast -> DVE mul
  yT[f,lq] = Wo-chunks.T @ obig (+bo)
"""

import numpy as np

import concourse.bass as bass
import concourse.tile as tile
from concourse import bacc, mybir
from concourse import bass_utils
from concourse.bass import ts, broadcast_tensor_aps
from concourse.masks import make_identity

F32 = mybir.dt.float32
F16 = mybir.dt.float16

B, L, F, H, D = 2, 2048, 1024, 16, 64
LQ = 512            # query rows per core
LK = 2048           # kv rows (full)
NCORES = 8
PAIRS = H // 2      # head pairs (one qT partition block each)
FCH = F // 128      # f contraction chunks
KCH = LK // 128     # lk chunks
NL = LK // LQ       # kv column blocks

_CACHED = {}
DEBUG = False


def build_nc(debug=False):
    nc = bacc.Bacc("TRN2", target_bir_lowering=False, debug=False,
                   num_devices=NCORES)
    dt_in = [
        ("xq_t", [FCH, 128, LQ], F16),         # [f, p, lq]
        ("xkv_t", [NL, FCH, 128, LQ], F16),    # [l, f, p, lq]
        ("mask_t", [KCH, 128, LQ], F16),       # [c, p, lq]
        ("wq", [FCH, 128, FCH, 128], F16),     # [j, p, f, m]
        ("wkv", [128, FCH, 128], F16),         # [p, f, m]
        ("wo", [FCH, 128, FCH, 128], F16),     # [fb, p, j, m]
        ("bqbo", [128, 2 * FCH], F32),         # cols 0:8 bq-blocks, 8:16 bo
        ("bkv", [2 * D], F32),
        ("cosq", [128, LQ], F32),
        ("sinq", [128, LQ], F32),
        ("cksk", [D, 2 * LK], F16),            # [p, (cos|sin)*lk]
    ]
    t = {name: nc.dram_tensor(name, shape, dt, kind="ExternalInput")
         for name, shape, dt in dt_in}
    yT = nc.dram_tensor("yT", [F, LQ], F32, kind="ExternalOutput")
    dbg = {}
    if debug:
        for name, shape, dt in [
            ("d_qrot0", [128, LQ], F16), ("d_ktop", [128, LK], F16),
            ("d_kvraw", [128, LK], F32), ("d_pt0", [128, 2, LQ], F16),
            ("d_pt15", [128, 2, LQ], F16), ("d_oa0", [128, LQ], F32),
            ("d_ob0", [128, LQ], F32), ("d_rec0", [65, LQ], F32),
            ("d_obig", [128, PAIRS, LQ], F16), ("d_vaug", [128, KCH, D + 1], F16),
        ]:
            dbg[name] = nc.dram_tensor(name, shape, dt, kind="ExternalOutput")

    with tile.TileContext(nc) as tc:
        with (
            tc.tile_pool(name="persist", bufs=1) as persist,
            tc.tile_pool(name="ptiles", bufs=3) as ptp,
            tc.tile_pool(name="small", bufs=1) as small,
            tc.tile_pool(name="xin", bufs=2) as xin,
            tc.tile_pool(name="wst", bufs=3) as wst,
            tc.tile_pool(name="ktmp", bufs=1) as ktmp,
            tc.tile_pool(name="rtp", bufs=2) as rtp,
            tc.tile_pool(name="qpool", bufs=2) as qpool,
            tc.tile_pool(name="recp", bufs=2) as recp,
            tc.tile_pool(name="yout", bufs=2) as yout,
            tc.tile_pool(name="psst", bufs=2, space="PSUM") as psst,
            tc.tile_pool(name="psacc", bufs=2, space="PSUM") as psacc,
            tc.tile_pool(name="pssm", bufs=2, space="PSUM") as pssm,
        ):
            # ---- input DMAs (priority order) ----
            mt = persist.tile([128, KCH, LQ], F16)    # maskT resident
            nc.gpsimd.dma_start(
                mt, t["mask_t"].ap().rearrange("c p q -> p c q"))

            wkv_sb = persist.tile([128, FCH, 128], F16)
            nc.sync.dma_start(wkv_sb, t["wkv"].ap())

            xq = persist.tile([128, FCH, LQ], F16)
            nc.sync.dma_start(xq, t["xq_t"].ap().rearrange("f p q -> p f q"))

            cq = persist.tile([128, LQ], F32)
            sq = persist.tile([128, LQ], F32)
            cksk = persist.tile([D, 2, LK], F16)
            nc.sync.dma_start(cq, t["cosq"].ap())
            nc.sync.dma_start(sq, t["sinq"].ap())
            nc.sync.dma_start(cksk,
                              t["cksk"].ap().rearrange("p (a l) -> p a l", a=2))
            ck = cksk[:, 0, :]
            sk = cksk[:, 1, :]

            bqbo = small.tile([128, 2 * FCH], F32, tag="bias")
            nc.sync.dma_start(bqbo, t["bqbo"].ap())
            bq_sb = bqbo[:, 0:FCH]
            bo_sb = bqbo[:, FCH:2 * FCH]
            bkv_sb = small.tile([128, 1], F32, tag="bias2")
            nc.sync.dma_start(bkv_sb, t["bkv"].ap().unsqueeze(1))

            wq_tiles = {}

            def prefetch_wq(j):
                wq_tiles[j] = wst.tile([128, FCH, 128], F16, tag="wq",
                                       name=f"wq_sb{j}")
                nc.sync.dma_start(wq_tiles[j], t["wq"].ap()[j])

            prefetch_wq(0)

            ones32 = small.tile([128, D], mybir.dt.float32r, tag="ones")
            nc.gpsimd.memset(ones32.bitcast(F32), 1.0)

            idt = small.tile([128, 128], F32, tag="ident")
            make_identity(nc, idt)
            # halves-swap permutation matrix: M[p, p-xor-32-within-head] = 1
            swp = small.tile([128, 128], F16, tag="swp")
            nc.gpsimd.memset(swp, 0.0)
            for o1, o2 in ((0, 32), (32, 0), (64, 96), (96, 64)):
                nc.gpsimd.affine_select(
                    out=swp[o1:o1 + 32, o2:o2 + 32],
                    in_=swp[o1:o1 + 32, o2:o2 + 32],
                    compare_op=mybir.AluOpType.not_equal, fill=1.0,
                    base=0, pattern=[[-1, 32]], channel_multiplier=1)

            # persistent SBUF state
            qrot = {}                                     # per-pair tiles
            ktop = persist.tile([128, LK], F16)           # k in rows 0:64
            kbot = persist.tile([128, LK], F16)           # k in rows 64:128
            vaug = persist.tile([128, KCH, D + 1], F16)   # V chunks + ones
            obig = persist.tile([128, PAIRS, LQ], F16)    # normalized O^T
            wo_sb = persist.tile([128, FCH, FCH, 128], F16)  # [p, fb, j, m]

            # ================= phase KV: projection + RoPE =================
            kvraw = persist.tile([128, LK], F32)
            for l in range(NL):
                xkv = xin.tile([128, FCH, LQ], F16, tag="x")
                nc.sync.dma_start(
                    xkv, t["xkv_t"].ap()[l].rearrange("f p q -> p f q"))
                pkv = pssm.tile([128, LQ], F32, tag="sm")
                for f in range(FCH):
                    nc.tensor.matmul(pkv, wkv_sb[:, f, :], xkv[:, f, :],
                                     start=(f == 0), stop=(f == FCH - 1))
                nc.vector.tensor_scalar_add(kvraw[:, ts(l, LQ)], pkv,
                                            bkv_sb[:, 0:1])

            # ---- RoPE on k: matmul-swap; kbot copy via DMA ----
            tmk = ktmp.tile([D, LK], F16, tag="ksin")
            nc.vector.tensor_mul(tmk, kvraw[0:64], sk)
            kc = ktmp.tile([D, LK], F16, tag="kcos")
            nc.vector.tensor_mul(kc, kvraw[0:64], ck)
            nc.vector.memset(ktop[64:128], 0.0)
            nc.vector.memset(kbot[0:64], 0.0)
            for l in range(NL):
                pswk = pssm.tile([128, LQ], F32, tag="sm")
                nc.tensor.matmul(pswk[0:64], swp[0:64, 0:64],
                                 tmk[:, ts(l, LQ)], start=True, stop=True)
                nc.vector.tensor_add(ktop[0:64, ts(l, LQ)],
                                     kc[:, ts(l, LQ)], pswk[0:64])
            nc.gpsimd.dma_start(kbot[64:128], ktop[0:64])

            # ---- V_aug: transpose v chunks, append ones column ----
            nc.vector.memset(vaug[:, :, D:D + 1], 1.0)
            for c in range(KCH):
                tp = pssm.tile([128, LQ], F32, tag="sm")
                nc.tensor.transpose(tp[:, 0:64], kvraw[64:128, ts(c, 128)],
                                    idt[64:128, 64:128])
                nc.vector.tensor_copy(vaug[:, c, 0:D], tp[:, 0:64])

            # ================= per-pair q-proj + RoPE =================
            def emit_qproj_rope(j):
                psq = pssm.tile([128, LQ], F32, tag="sm")
                wq_j = wq_tiles.pop(j)
                for f in range(FCH):
                    nc.tensor.matmul(psq, wq_j[:, f, :], xq[:, f, :],
                                     start=(f == 0), stop=(f == FCH - 1))
                # tmq = (psq + bq) * sin ; qc = (psq + bq) * cos
                tmq = rtp.tile([128, LQ], F16, tag="qsin")
                nc.vector.scalar_tensor_tensor(
                    out=tmq, in0=psq, scalar=bq_sb[:, j:j + 1], in1=sq,
                    op0=mybir.AluOpType.add, op1=mybir.AluOpType.mult)
                psw = pssm.tile([128, LQ], F32, tag="sm")
                nc.tensor.matmul(psw, swp, tmq, start=True, stop=True)
                qc = rtp.tile([128, LQ], F32, tag="qcos")
                nc.vector.scalar_tensor_tensor(
                    out=qc, in0=psq, scalar=bq_sb[:, j:j + 1], in1=cq,
                    op0=mybir.AluOpType.add, op1=mybir.AluOpType.mult)
                qrot[j] = qpool.tile([128, LQ], F16, tag="qrot",
                                     name=f"qrot{j}")
                nc.vector.tensor_add(qrot[j], qc, psw)

            emit_qproj_rope(0)
            prefetch_wq(1)
            if debug:
                nc.sync.dma_start(dbg["d_qrot0"].ap(), qrot[0])

            # ================= attention pair loop =================
            def emit_normalize(j, oa, ob):
                # row 64 of oa/ob = softmax denominator Z
                for tt, op in ((0, oa), (1, ob)):
                    rec = recp.tile([65, LQ], mybir.dt.float32r, tag="rec")
                    with nc.allow_low_precision(reason="f32r recip to matmul"):
                        nc.vector.reciprocal(rec[64:65, :], op[D:D + 1, :])
                    if debug and j == 0 and tt == 0:
                        nc.sync.dma_start(dbg["d_rec0"].ap(),
                                          rec.bitcast(F32))
                    rbp = pssm.tile([128, LQ], F32, tag="sm")
                    nc.tensor.matmul(rbp[0:D, :], ones32[64:65, :],
                                     rec[64:65, :], start=True, stop=True)
                    rbs = recp.tile([D, LQ], F32, tag="rbs")
                    nc.vector.tensor_copy(rbs, rbp[0:D, :])
                    if tt == 0:
                        nc.vector.tensor_mul(obig[0:D, j, :], op[0:D, :], rbs)
                    else:
                        osb = recp.tile([D, LQ], F16, tag="osb")
                        nc.vector.tensor_mul(osb, op[0:D, :], rbs)
                        nc.gpsimd.dma_start(obig[64:128, j, :], osb)

            for j in range(PAIRS):
                oa = psacc.tile([128, LQ], F32, tag="acc")
                ob = psacc.tile([128, LQ], F32, tag="acc")
                for c in range(KCH):
                    st = psst.tile([128, 2, LQ], F32, tag="st")
                    nc.tensor.matmul(st[:, 0, :], ktop[:, ts(c, 128)],
                                     qrot[j], start=True, stop=True)
                    nc.tensor.matmul(st[:, 1, :], kbot[:, ts(c, 128)],
                                     qrot[j], start=True, stop=True)
                    pt = ptp.tile([128, 2, LQ], F16, tag="p")
                    nc.scalar.activation(pt, st,
                                         mybir.ActivationFunctionType.Exp)
                    # mask multiply, mostly on GpSimd to keep DVE headroom
                    p_b, m_b = broadcast_tensor_aps(pt, mt[:, c:c + 1, :])
                    eng = nc.vector if c % 3 == 0 else nc.gpsimd
                    eng.tensor_tensor(out=p_b, in0=p_b, in1=m_b,
                                      op=mybir.AluOpType.mult)
                    if debug and j == 0 and c == 0:
                        nc.sync.dma_start(dbg["d_pt0"].ap(), pt)
                    if debug and j == 0 and c == KCH - 1:
                        nc.sync.dma_start(dbg["d_pt15"].ap(), pt)
                    nc.tensor.matmul(oa[0:D + 1, :], vaug[:, c, :],
                                     pt[:, 0, :], start=(c == 0),
                                     stop=(c == KCH - 1))
                    nc.tensor.matmul(ob[0:D + 1, :], vaug[:, c, :],
                                     pt[:, 1, :], start=(c == 0),
                                     stop=(c == KCH - 1))
                    if c == 4 and j + 2 < PAIRS:
                        prefetch_wq(j + 2)
                    if c == 5 and j < FCH:
                        # prefetch one wo block per pair into wo_sb
                        nc.sync.dma_start(wo_sb[:, j, :, :], t["wo"].ap()[j])
                    if c == 6 and j + 1 < PAIRS:
                        emit_qproj_rope(j + 1)
                if debug and j == 0:
                    dcp_a = yout.tile([128, LQ], F32, tag="y")
                    nc.vector.tensor_copy(dcp_a, oa)
                    nc.sync.dma_start(dbg["d_oa0"].ap(), dcp_a)
                    dcp_b = yout.tile([128, LQ], F32, tag="y")
                    nc.vector.tensor_copy(dcp_b, ob)
                    nc.sync.dma_start(dbg["d_ob0"].ap(), dcp_b)
                emit_normalize(j, oa, ob)

            if debug:
                nc.sync.dma_start(dbg["d_ktop"].ap(), ktop)
                nc.sync.dma_start(dbg["d_kvraw"].ap(), kvraw)
                nc.sync.dma_start(dbg["d_obig"].ap(), obig)
                nc.sync.dma_start(dbg["d_vaug"].ap(), vaug)

            # ================= phase D: output projection =================
            for fb in range(FCH):
                psy = psacc.tile([128, LQ], F32, tag="acc")
                for j in range(FCH):
                    nc.tensor.matmul(psy, wo_sb[:, fb, j, :], obig[:, j, :],
                                     start=(j == 0), stop=(j == FCH - 1))
                ysb = yout.tile([128, LQ], F32, tag="y")
                nc.vector.tensor_scalar_add(ysb, psy, bo_sb[:, fb:fb + 1])
                nc.sync.dma_start(yT.ap()[ts(fb, 128), :], ysb)

    nc.compile()
    return nc


def _tables():
    """RoPE tables in halves-permuted basis: rows i (even-half) hold +sin,
    rows 32+i (odd-half) hold -sin (for the tmp-then-swap formulation)."""
    inv_freq = 1.0 / (10000.0 ** (np.arange(0, D, 2, dtype=np.float64) / D))
    ang = np.outer(inv_freq, np.arange(L, dtype=np.float64))  # [32, L]
    cos = np.cos(ang).astype(np.float32)
    sin = np.sin(ang).astype(np.float32)
    cos64 = np.concatenate([cos, cos], axis=0)                # [64, L]
    sin_sgn = np.concatenate([sin, -sin], axis=0)             # [64, L]
    return cos64, sin_sgn


def _prep_weights(Wq, bq, Wk, bk, Wv, bv, Wo, bo):
    perm = np.concatenate([np.arange(0, D, 2), np.arange(1, D, 2)])
    WqP = np.asarray(Wq, dtype=np.float32)[:, :, perm].reshape(F, H * D)
    bqP = np.asarray(bq, dtype=np.float32)[:, perm].reshape(H * D)
    WkP = np.asarray(Wk, dtype=np.float32)[:, perm]
    bkP = np.asarray(bk, dtype=np.float32)[perm]
    Wkv = np.concatenate([WkP, np.asarray(Wv, dtype=np.float32)], axis=1)
    bkv = np.concatenate([bkP, np.asarray(bv, dtype=np.float32)])
    WoR = np.asarray(Wo, dtype=np.float32).reshape(H * D, F)
    bo_ = np.asarray(bo, dtype=np.float32)

    wq_pre = np.ascontiguousarray(
        WqP.reshape(FCH, 128, FCH, 128).transpose(2, 1, 0, 3)
    ).astype(np.float16)
    wkv_pre = np.ascontiguousarray(
        Wkv.reshape(FCH, 128, 128).transpose(1, 0, 2)).astype(np.float16)
    wo_pre = np.ascontiguousarray(
        WoR.reshape(FCH, 128, FCH, 128).transpose(2, 1, 0, 3)
    ).astype(np.float16)
    bqbo = np.ascontiguousarray(np.concatenate(
        [bqP.reshape(FCH, 128).T, bo_.reshape(FCH, 128).T], axis=1))
    return wq_pre, wkv_pre, wo_pre, bqbo, bkv


def kernel(inputs_q, inputs_kv, mask, Wq, bq, Wk, bk, Wv, bv, Wo, bo):
    if "nc" not in _CACHED:
        _CACHED["nc"] = build_nc(debug=DEBUG)
    nc = _CACHED["nc"]

    wq_pre, wkv_pre, wo_pre, bqbo, bkv = _prep_weights(
        Wq, bq, Wk, bk, Wv, bv, Wo, bo)

    cos64, sin_sgn = _tables()
    scale = 1.0 / np.sqrt(np.float32(D))
    cksk = np.ascontiguousarray(
        np.concatenate([cos64, sin_sgn], axis=1)).astype(np.float16)
    cosq_full = np.tile(cos64 * scale, (2, 1))         # [128, L]
    sinq_full = np.tile(sin_sgn * scale, (2, 1))

    xq = np.asarray(inputs_q, dtype=np.float32)
    xkv = np.asarray(inputs_kv, dtype=np.float32)
    mk = np.asarray(mask)

    in_maps = []
    for core in range(NCORES):
        b = core // 4
        qs = (core % 4) * LQ
        xq_t = np.ascontiguousarray(
            xq[b, qs:qs + LQ, :].T.reshape(FCH, 128, LQ)).astype(np.float16)
        xkv_t = np.ascontiguousarray(
            xkv[b].T.reshape(FCH, 128, NL, LQ).transpose(2, 0, 1, 3)
        ).astype(np.float16)
        mask_t = np.ascontiguousarray(
            mk[b, 0, qs:qs + LQ, :].T.reshape(KCH, 128, LQ)
            .astype(np.float16))
        in_maps.append({
            "xq_t": xq_t,
            "xkv_t": xkv_t,
            "mask_t": mask_t,
            "wq": wq_pre,
            "wkv": wkv_pre,
            "wo": wo_pre,
            "bqbo": bqbo,
            "bkv": bkv,
            "cosq": np.ascontiguousarray(cosq_full[:, qs:qs + LQ]),
            "sinq": np.ascontiguousarray(sinq_full[:, qs:qs + LQ]),
            "cksk": cksk,
        })

    res = bass_utils.run_bass_kernel_spmd(nc, in_maps,
                                          core_ids=list(range(NCORES)))
    _CACHED["last_results"] = res
    _CACHED["last_maps"] = in_maps

    out = np.empty((B, L, F), dtype=np.float32)
    for core in range(NCORES):
        b = core // 4
        qs = (core % 4) * LQ
        out[b, qs:qs + LQ, :] = res.results[core]["yT"].T
    return out


# revision 32
# speedup vs baseline: 1.0037x; 1.0037x over previous
"""MQA attention (B=2, Lq=Lkv=2048, F=1024, H=16, D=64) on 8 TRN2 cores.

Sharding: core = (batch, query-block-of-512). Each core computes its full
output rows (all 16 heads + output projection) -> no collectives; host
concatenates per-core yT slabs.

v2 (pipelined): fp16 inputs/weights, KV projection first, then an
ACT-bound software pipeline over 8 head-pairs where pair j+1's
q-projection + RoPE are emitted inside pair j's attention chunk loop.
Mask multiplies alternate DVE/GpSimd; softmax denominators use
reciprocal_approx_fast + DMA partition-broadcast off the critical path.

Per-core dataflow:
  kvT[kd|vd,lk] = Wkv.T @ xkvT          (fp16 x, fp16 w, fp32 psum)
  RoPE in halves-permuted basis (host permutes Wq/Wk columns):
  x_rot = x*cos + Swap @ (x*sin_signed), Swap = permutation on the PE.
  qT[hd,lq] per pair = Wq'.T @ xqT  (+RoPE, scaled 1/sqrt(D) via tables)
  S^T[lk,lq] per head = k-chunk.T @ qT   (fp16, zero-padded K=128)
  P^T = exp(S^T) * maskT  (ACT exp -> fp16; mask mul on DVE/GpSimd)
  O_aug^T = V_aug-chunk.T @ P^T  (ones column -> row 64 = denominator)
  normalize: reciprocal_approx_fast(Z) -> DMA broadcast -> DVE mul
  yT[f,lq] = Wo-chunks.T @ obig (+bo)
"""

import numpy as np

import concourse.bass as bass
import concourse.tile as tile
from concourse import bacc, mybir
from concourse import bass_utils
from concourse.bass import ts, broadcast_tensor_aps
from concourse.masks import make_identity

F32 = mybir.dt.float32
F16 = mybir.dt.float16

B, L, F, H, D = 2, 2048, 1024, 16, 64
LQ = 512            # query rows per core
LK = 2048           # kv rows (full)
NCORES = 8
PAIRS = H // 2      # head pairs (one qT partition block each)
FCH = F // 128      # f contraction chunks
KCH = LK // 128     # lk chunks
NL = LK // LQ       # kv column blocks

_CACHED = {}
DEBUG = False


def build_nc(debug=False):
    nc = bacc.Bacc("TRN2", target_bir_lowering=False, debug=False,
                   num_devices=NCORES)
    dt_in = [
        ("xq_t", [FCH, 128, LQ], F16),         # [f, p, lq]
        ("xkv_t", [NL, FCH, 128, LQ], F16),    # [l, f, p, lq]
        ("mask_t", [KCH, 128, 2, LQ], F16),    # [c, p, tt, lq] (doubled)
        ("wq", [FCH, 128, FCH, 128], F16),     # [j, p, f, m]
        ("wkv", [128, FCH, 128], F16),         # [p, f, m]
        ("wo", [FCH, 128, FCH, 128], F16),     # [fb, p, j, m]
        ("bqbo", [128, 2 * FCH], F32),         # cols 0:8 bq-blocks, 8:16 bo
        ("bkv", [2 * D], F32),
        ("cosq", [128, LQ], F32),
        ("sinq", [128, LQ], F32),
        ("cksk", [D, 2 * LK], F16),            # [p, (cos|sin)*lk]
    ]
    t = {name: nc.dram_tensor(name, shape, dt, kind="ExternalInput")
         for name, shape, dt in dt_in}
    yT = nc.dram_tensor("yT", [F, LQ], F32, kind="ExternalOutput")
    dbg = {}
    if debug:
        for name, shape, dt in [
            ("d_qrot0", [128, LQ], F16), ("d_ktop", [128, LK], F16),
            ("d_kvraw", [128, LK], F32), ("d_pt0", [128, 2, LQ], F16),
            ("d_pt15", [128, 2, LQ], F16), ("d_oa0", [128, LQ], F32),
            ("d_ob0", [128, LQ], F32), ("d_rec0", [65, LQ], F32),
            ("d_obig", [128, PAIRS, LQ], F16), ("d_vaug", [128, KCH, D + 1], F16),
        ]:
            dbg[name] = nc.dram_tensor(name, shape, dt, kind="ExternalOutput")

    with tile.TileContext(nc) as tc:
        with (
            tc.tile_pool(name="persist", bufs=1) as persist,
            tc.tile_pool(name="ptiles", bufs=3) as ptp,
            tc.tile_pool(name="small", bufs=1) as small,
            tc.tile_pool(name="xin", bufs=2) as xin,
            tc.tile_pool(name="wst", bufs=3) as wst,
            tc.tile_pool(name="ktmp", bufs=1) as ktmp,
            tc.tile_pool(name="rtp", bufs=2) as rtp,
            tc.tile_pool(name="qpool", bufs=2) as qpool,
            tc.tile_pool(name="recp", bufs=2) as recp,
            tc.tile_pool(name="yout", bufs=2) as yout,
            tc.tile_pool(name="psst", bufs=2, space="PSUM") as psst,
            tc.tile_pool(name="psacc", bufs=2, space="PSUM") as psacc,
            tc.tile_pool(name="pssm", bufs=2, space="PSUM") as pssm,
        ):
            # ---- input DMAs (priority order) ----
            wkv_sb = persist.tile([128, FCH, 128], F16)
            nc.sync.dma_start(wkv_sb, t["wkv"].ap())

            mt = persist.tile([128, KCH, 2, LQ], F16)    # maskT resident
            nc.gpsimd.dma_start(
                mt, t["mask_t"].ap().rearrange("c p t q -> p c t q"))

            xq = persist.tile([128, FCH, LQ], F16)
            nc.scalar.dma_start(xq, t["xq_t"].ap().rearrange("f p q -> p f q"))

            cq = persist.tile([128, LQ], F32)
            sq = persist.tile([128, LQ], F32)
            cksk = persist.tile([D, 2, LK], F16)
            nc.scalar.dma_start(cq, t["cosq"].ap())
            nc.scalar.dma_start(sq, t["sinq"].ap())
            nc.scalar.dma_start(cksk,
                                t["cksk"].ap().rearrange("p (a l) -> p a l",
                                                         a=2))
            ck = cksk[:, 0, :]
            sk = cksk[:, 1, :]

            bqbo = small.tile([128, 2 * FCH], F32, tag="bias")
            nc.scalar.dma_start(bqbo, t["bqbo"].ap())
            bq_sb = bqbo[:, 0:FCH]
            bo_sb = bqbo[:, FCH:2 * FCH]
            bkv_sb = small.tile([128, 1], F32, tag="bias2")
            nc.scalar.dma_start(bkv_sb, t["bkv"].ap().unsqueeze(1))

            wq_tiles = {}

            def prefetch_wq(j):
                wq_tiles[j] = wst.tile([128, FCH, 128], F16, tag="wq",
                                       name=f"wq_sb{j}")
                nc.sync.dma_start(wq_tiles[j], t["wq"].ap()[j])

            prefetch_wq(0)

            ones32 = small.tile([128, D], mybir.dt.float32r, tag="ones")
            nc.gpsimd.memset(ones32.bitcast(F32), 1.0)

            idt = small.tile([128, 128], F32, tag="ident")
            make_identity(nc, idt)
            # halves-swap permutation matrix: M[p, p-xor-32-within-head] = 1
            swp = small.tile([128, 128], F16, tag="swp")
            nc.gpsimd.memset(swp, 0.0)
            for o1, o2 in ((0, 32), (32, 0), (64, 96), (96, 64)):
                nc.gpsimd.affine_select(
                    out=swp[o1:o1 + 32, o2:o2 + 32],
                    in_=swp[o1:o1 + 32, o2:o2 + 32],
                    compare_op=mybir.AluOpType.not_equal, fill=1.0,
                    base=0, pattern=[[-1, 32]], channel_multiplier=1)

            # persistent SBUF state
            qrot = {}                                     # per-pair tiles
            ktop = persist.tile([128, LK], F16)           # k in rows 0:64
            kbot = persist.tile([128, LK], F16)           # k in rows 64:128
            vaug = persist.tile([128, KCH, D + 1], F16)   # V chunks + ones
            obig = persist.tile([128, PAIRS, LQ], F16)    # normalized O^T
            wo_sb = persist.tile([128, FCH, FCH, 128], F16)  # [p, fb, j, m]

            # ================= phase KV: projection + RoPE =================
            kvraw = persist.tile([128, LK], F32)
            for l in range(NL):
                xkv = xin.tile([128, FCH, LQ], F16, tag="x")
                nc.sync.dma_start(
                    xkv, t["xkv_t"].ap()[l].rearrange("f p q -> p f q"))
                pkv = pssm.tile([128, LQ], F32, tag="sm")
                for f in range(FCH):
                    nc.tensor.matmul(pkv, wkv_sb[:, f, :], xkv[:, f, :],
                                     start=(f == 0), stop=(f == FCH - 1))
                nc.vector.tensor_scalar_add(kvraw[:, ts(l, LQ)], pkv,
                                            bkv_sb[:, 0:1])

            # ---- RoPE on k: matmul-swap; kbot copy via DMA ----
            tmk = ktmp.tile([D, LK], F16, tag="ksin")
            nc.vector.tensor_mul(tmk, kvraw[0:64], sk)
            kc = ktmp.tile([D, LK], F16, tag="kcos")
            nc.vector.tensor_mul(kc, kvraw[0:64], ck)
            nc.vector.memset(ktop[64:128], 0.0)
            nc.vector.memset(kbot[0:64], 0.0)
            for l in range(NL):
                pswk = pssm.tile([128, LQ], F32, tag="sm")
                nc.tensor.matmul(pswk[0:64], swp[0:64, 0:64],
                                 tmk[:, ts(l, LQ)], start=True, stop=True)
                nc.vector.tensor_add(ktop[0:64, ts(l, LQ)],
                                     kc[:, ts(l, LQ)], pswk[0:64])
            nc.gpsimd.dma_start(kbot[64:128], ktop[0:64])

            # ---- V_aug: transpose v chunks, append ones column ----
            nc.vector.memset(vaug[:, :, D:D + 1], 1.0)
            for c in range(KCH):
                tp = pssm.tile([128, LQ], F32, tag="sm")
                nc.tensor.transpose(tp[:, 0:64], kvraw[64:128, ts(c, 128)],
                                    idt[64:128, 64:128])
                nc.vector.tensor_copy(vaug[:, c, 0:D], tp[:, 0:64])

            # ================= per-pair q-proj + RoPE =================
            def emit_qproj_rope(j):
                psq = pssm.tile([128, LQ], F32, tag="sm")
                wq_j = wq_tiles.pop(j)
                for f in range(FCH):
                    nc.tensor.matmul(psq, wq_j[:, f, :], xq[:, f, :],
                                     start=(f == 0), stop=(f == FCH - 1))
                # tmq = (psq + bq) * sin ; qc = (psq + bq) * cos
                tmq = rtp.tile([128, LQ], F16, tag="qsin")
                nc.vector.scalar_tensor_tensor(
                    out=tmq, in0=psq, scalar=bq_sb[:, j:j + 1], in1=sq,
                    op0=mybir.AluOpType.add, op1=mybir.AluOpType.mult)
                psw = pssm.tile([128, LQ], F32, tag="sm")
                nc.tensor.matmul(psw, swp, tmq, start=True, stop=True)
                qc = rtp.tile([128, LQ], F32, tag="qcos")
                nc.vector.scalar_tensor_tensor(
                    out=qc, in0=psq, scalar=bq_sb[:, j:j + 1], in1=cq,
                    op0=mybir.AluOpType.add, op1=mybir.AluOpType.mult)
                qrot[j] = qpool.tile([128, LQ], F16, tag="qrot",
                                     name=f"qrot{j}")
                nc.vector.tensor_add(qrot[j], qc, psw)

            emit_qproj_rope(0)
            prefetch_wq(1)
            if debug:
                nc.sync.dma_start(dbg["d_qrot0"].ap(), qrot[0])

            # ================= attention pair loop =================
            def emit_normalize(j, oa, ob):
                # row 64 of oa/ob = softmax denominator Z
                for tt, op in ((0, oa), (1, ob)):
                    rec = recp.tile([65, LQ], mybir.dt.float32r, tag="rec")
                    with nc.allow_low_precision(reason="f32r recip to matmul"):
                        nc.vector.reciprocal(rec[64:65, :], op[D:D + 1, :])
                    if debug and j == 0 and tt == 0:
                        nc.sync.dma_start(dbg["d_rec0"].ap(),
                                          rec.bitcast(F32))
                    rbp = pssm.tile([128, LQ], F32, tag="sm")
                    nc.tensor.matmul(rbp[0:D, :], ones32[64:65, :],
                                     rec[64:65, :], start=True, stop=True)
                    rbs = recp.tile([D, LQ], F32, tag="rbs")
                    nc.vector.tensor_copy(rbs, rbp[0:D, :])
                    if tt == 0:
                        nc.vector.tensor_mul(obig[0:D, j, :], op[0:D, :], rbs)
                    else:
                        osb = recp.tile([D, LQ], F16, tag="osb")
                        nc.vector.tensor_mul(osb, op[0:D, :], rbs)
                        nc.gpsimd.dma_start(obig[64:128, j, :], osb)

            for j in range(PAIRS):
                oa = psacc.tile([128, LQ], F32, tag="acc")
                ob = psacc.tile([128, LQ], F32, tag="acc")
                for c in range(KCH):
                    st = psst.tile([128, 2, LQ], F32, tag="st")
                    nc.tensor.matmul(st[:, 0, :], ktop[:, ts(c, 128)],
                                     qrot[j], start=True, stop=True)
                    nc.tensor.matmul(st[:, 1, :], kbot[:, ts(c, 128)],
                                     qrot[j], start=True, stop=True)
                    pt = ptp.tile([128, 2, LQ], F16, tag="p")
                    nc.scalar.activation(pt, st,
                                         mybir.ActivationFunctionType.Exp)
                    # mask multiply: split DVE / GpSimd
                    mm = mt[:, c, :, :]
                    eng = nc.vector if c % 2 == 0 else nc.gpsimd
                    eng.tensor_tensor(out=pt, in0=pt, in1=mm,
                                      op=mybir.AluOpType.mult)
                    if debug and j == 0 and c == 0:
                        nc.sync.dma_start(dbg["d_pt0"].ap(), pt)
                    if debug and j == 0 and c == KCH - 1:
                        nc.sync.dma_start(dbg["d_pt15"].ap(), pt)
                    nc.tensor.matmul(oa[0:D + 1, :], vaug[:, c, :],
                                     pt[:, 0, :], start=(c == 0),
                                     stop=(c == KCH - 1))
                    nc.tensor.matmul(ob[0:D + 1, :], vaug[:, c, :],
                                     pt[:, 1, :], start=(c == 0),
                                     stop=(c == KCH - 1))
                    if c == 4 and j + 2 < PAIRS:
                        prefetch_wq(j + 2)
                    if c == 5 and j < FCH:
                        # prefetch one wo block per pair into wo_sb
                        nc.sync.dma_start(wo_sb[:, j, :, :], t["wo"].ap()[j])
                    if c == 6 and j + 1 < PAIRS:
                        emit_qproj_rope(j + 1)
                if debug and j == 0:
                    dcp_a = yout.tile([128, LQ], F32, tag="y")
                    nc.vector.tensor_copy(dcp_a, oa)
                    nc.sync.dma_start(dbg["d_oa0"].ap(), dcp_a)
                    dcp_b = yout.tile([128, LQ], F32, tag="y")
                    nc.vector.tensor_copy(dcp_b, ob)
                    nc.sync.dma_start(dbg["d_ob0"].ap(), dcp_b)
                emit_normalize(j, oa, ob)

            if debug:
                nc.sync.dma_start(dbg["d_ktop"].ap(), ktop)
                nc.sync.dma_start(dbg["d_kvraw"].ap(), kvraw)
                nc.sync.dma_start(dbg["d_obig"].ap(), obig)
                nc.sync.dma_start(dbg["d_vaug"].ap(), vaug)

            # ================= phase D: output projection =================
            for fb in range(FCH):
                psy = psacc.tile([128, LQ], F32, tag="acc")
                for j in range(FCH):
                    nc.tensor.matmul(psy, wo_sb[:, fb, j, :], obig[:, j, :],
                                     start=(j == 0), stop=(j == FCH - 1))
                ysb = yout.tile([128, LQ], F32, tag="y")
                nc.vector.tensor_scalar_add(ysb, psy, bo_sb[:, fb:fb + 1])
                nc.sync.dma_start(yT.ap()[ts(fb, 128), :], ysb)

    nc.compile()
    return nc


def _tables():
    """RoPE tables in halves-permuted basis: rows i (even-half) hold +sin,
    rows 32+i (odd-half) hold -sin (for the tmp-then-swap formulation)."""
    inv_freq = 1.0 / (10000.0 ** (np.arange(0, D, 2, dtype=np.float64) / D))
    ang = np.outer(inv_freq, np.arange(L, dtype=np.float64))  # [32, L]
    cos = np.cos(ang).astype(np.float32)
    sin = np.sin(ang).astype(np.float32)
    cos64 = np.concatenate([cos, cos], axis=0)                # [64, L]
    sin_sgn = np.concatenate([sin, -sin], axis=0)             # [64, L]
    return cos64, sin_sgn


def _prep_weights(Wq, bq, Wk, bk, Wv, bv, Wo, bo):
    perm = np.concatenate([np.arange(0, D, 2), np.arange(1, D, 2)])
    WqP = np.asarray(Wq, dtype=np.float32)[:, :, perm].reshape(F, H * D)
    bqP = np.asarray(bq, dtype=np.float32)[:, perm].reshape(H * D)
    WkP = np.asarray(Wk, dtype=np.float32)[:, perm]
    bkP = np.asarray(bk, dtype=np.float32)[perm]
    Wkv = np.concatenate([WkP, np.asarray(Wv, dtype=np.float32)], axis=1)
    bkv = np.concatenate([bkP, np.asarray(bv, dtype=np.float32)])
    WoR = np.asarray(Wo, dtype=np.float32).reshape(H * D, F)
    bo_ = np.asarray(bo, dtype=np.float32)

    wq_pre = np.ascontiguousarray(
        WqP.reshape(FCH, 128, FCH, 128).transpose(2, 1, 0, 3)
    ).astype(np.float16)
    wkv_pre = np.ascontiguousarray(
        Wkv.reshape(FCH, 128, 128).transpose(1, 0, 2)).astype(np.float16)
    wo_pre = np.ascontiguousarray(
        WoR.reshape(FCH, 128, FCH, 128).transpose(2, 1, 0, 3)
    ).astype(np.float16)
    bqbo = np.ascontiguousarray(np.concatenate(
        [bqP.reshape(FCH, 128).T, bo_.reshape(FCH, 128).T], axis=1))
    return wq_pre, wkv_pre, wo_pre, bqbo, bkv


def kernel(inputs_q, inputs_kv, mask, Wq, bq, Wk, bk, Wv, bv, Wo, bo):
    if "nc" not in _CACHED:
        _CACHED["nc"] = build_nc(debug=DEBUG)
    nc = _CACHED["nc"]

    wq_pre, wkv_pre, wo_pre, bqbo, bkv = _prep_weights(
        Wq, bq, Wk, bk, Wv, bv, Wo, bo)

    cos64, sin_sgn = _tables()
    scale = 1.0 / np.sqrt(np.float32(D))
    cksk = np.ascontiguousarray(
        np.concatenate([cos64, sin_sgn], axis=1)).astype(np.float16)
    cosq_full = np.tile(cos64 * scale, (2, 1))         # [128, L]
    sinq_full = np.tile(sin_sgn * scale, (2, 1))

    xq = np.asarray(inputs_q, dtype=np.float32)
    xkv = np.asarray(inputs_kv, dtype=np.float32)
    mk = np.asarray(mask)

    in_maps = []
    for core in range(NCORES):
        b = core // 4
        qs = (core % 4) * LQ
        xq_t = np.ascontiguousarray(
            xq[b, qs:qs + LQ, :].T.reshape(FCH, 128, LQ)).astype(np.float16)
        xkv_t = np.ascontiguousarray(
            xkv[b].T.reshape(FCH, 128, NL, LQ).transpose(2, 0, 1, 3)
        ).astype(np.float16)
        mask_1 = mk[b, 0, qs:qs + LQ, :].T.reshape(KCH, 128, LQ)
        mask_t = np.ascontiguousarray(np.broadcast_to(
            mask_1[:, :, None, :], (KCH, 128, 2, LQ))).astype(np.float16)
        in_maps.append({
            "xq_t": xq_t,
            "xkv_t": xkv_t,
            "mask_t": mask_t,
            "wq": wq_pre,
            "wkv": wkv_pre,
            "wo": wo_pre,
            "bqbo": bqbo,
            "bkv": bkv,
            "cosq": np.ascontiguousarray(cosq_full[:, qs:qs + LQ]),
            "sinq": np.ascontiguousarray(sinq_full[:, qs:qs + LQ]),
            "cksk": cksk,
        })

    res = bass_utils.run_bass_kernel_spmd(nc, in_maps,
                                          core_ids=list(range(NCORES)))
    _CACHED["last_results"] = res
    _CACHED["last_maps"] = in_maps

    out = np.empty((B, L, F), dtype=np.float32)
    for core in range(NCORES):
        b = core // 4
        qs = (core % 4) * LQ
        out[b, qs:qs + LQ, :] = res.results[core]["yT"].T
    return out
